# revision 23
# baseline (speedup 1.0000x reference)
"""Trainium2 Bass kernel for the DiscretizedDPLRSSMBlock problem.

Computes, for h, x of shape [4096, 4096] (batch, hidden):

    out = h + (h * a_diag + (h @ q_vec) @ p_vec.T) + x @ b_mat        (DELTA = 1.0)
        = h * (1 + a_diag) + (h @ q_vec) @ p_vec.T + x @ b_mat

Sharding: data-parallel over the batch axis across 8 NeuronCores (512 rows
per core); a_diag/p_vec/q_vec/b_mat replicated.

Per-core kernel works in a transposed layout (hidden on partitions):
    outT[n, m] = sum_k B[k, n] * xT[k, m]        (x @ B; fp8e4 DoubleRow
                                                  matmuls, 2 k-subtiles per
                                                  instruction, 2x throughput)
               + sum_r p[n, r] * hqT[r, m]       (rank-4 term)
               + (1 + a[n]) * hT[n, m]           (per-partition scalar on DVE)

Precision plan (rel-err gate is 2e-2; measured ~1.78e-2):
  - x*2^5 and B*2^13 are cast to fp8e4 on the host (all values stay inside
    TRN e4m3's +-240 normal range); the epilogue multiplies the PSUM
    accumulator by 2^-18 (exact power of two) before adding h*(1+a).
  - hqT = q^T @ hT runs on a second fp8 copy of h (h*2^5) against q*2^12,
    column-tiled 4-wide (tile_position col groups; partials land on
    partition bases 0/32/64/96 of one PSUM bank).  The replicated-pT rows
    of the rank-4 stationary operand sum the partials inside the rank-4
    matmul itself.  The rank-4 term is ~2% of the output, so fp8 h/q
    quantization there is negligible.
  - h also arrives as bf16 -- streamed lazily, one chunk per two groups --
    for the h*(1+a) epilogue term, which needs full precision.
  - Output is fp16 (upcast on host).

DMA: one hand-ordered stream on the Sync HWDGE ring feeds the PE exactly
in consumption order (b group chunks interleaved with the x lead-in, the
fp8 h, and the lazy bf16 h); output tiles ride the Scalar ring.  All
operand tensors are host-pre-shuffled into partition-major tile layouts so
every chunk DMA reads contiguous 8-16KB runs per partition row.
"""

import numpy as np
import ml_dtypes

import concourse.mybir as mybir
import concourse.tile as tile
from concourse import bacc
from concourse.bass_utils import run_bass_kernel_spmd

HIDDEN = 4096
BATCH = 4096
RANK = 4
N_CORES = 8
MB = BATCH // N_CORES  # 512 batch rows per core
P = 128
KT = HIDDEN // P       # 32 contraction tiles
NT = HIDDEN // P       # 32 output row tiles (hidden)
NGROUP = NT // 8       # 4 n-tiles per b-column streaming group (512 cols)
CH = KT // 4           # 8 k-tiles per bf16 hT chunk (1MB)
N_GROUPS = NT // NGROUP

SX = 2.0 ** 5          # host fp8 scale on x   (|x*32|   <= ~174 < 240)
SBC = 2.0 ** 13        # host fp8 scale on b   (|b*8192| <= 128  < 240)
SH = 2.0 ** 5          # host fp8 scale on h (hq path)
SQ = 2.0 ** 12         # host fp8 scale on q   (|q*4096| <= ~91  < 240)
INV_S = float(2.0 ** -18)   # exact de-scale applied to PSUM
HQ_FIX = float(2.0 ** 1)    # hq_ps carries 2^17; mains carry 2^18

BF16 = mybir.dt.bfloat16
FP8 = mybir.dt.float8e4
F16 = mybir.dt.float16
F32 = mybir.dt.float32
DR = mybir.MatmulPerfMode.DoubleRow

# k-tile chunking.  Group 0 (and x) use small lead-in chunks so the first
# matmuls start early; steady-state b chunks are ~1MB fp8.
CHUNKS_LEAD = [(0, 4), (4, 12), (16, 16)]
CHUNKS_MAIN = [(0, 16), (16, 16)]


def build_bass():
    """Build the single-core Tile program (same program runs SPMD on all 8)."""
    nc = bacc.Bacc("TRN2", target_bir_lowering=False, debug=False)

    b = nc.dram_tensor("b", [P, N_GROUPS * KT, NGROUP * P], FP8,
                       kind="ExternalInput")
    xT = nc.dram_tensor("xT", [P, KT, MB], FP8, kind="ExternalInput")
    hT = nc.dram_tensor("hT", [P, KT, MB], BF16, kind="ExternalInput")
    h8 = nc.dram_tensor("h8", [P, KT, MB], FP8, kind="ExternalInput")
    q = nc.dram_tensor("q", [P, KT, RANK], FP8, kind="ExternalInput")
    pT = nc.dram_tensor("pT", [P, HIDDEN], BF16, kind="ExternalInput")
    a_r = nc.dram_tensor("a_r", [P, NT], F32, kind="ExternalInput")
    outT = nc.dram_tensor("outT", [HIDDEN, MB], F16, kind="ExternalOutput")

    with (
        tile.TileContext(nc) as tc,
        tc.tile_pool(name="const", bufs=1) as cpool,
        tc.tile_pool(name="bcols", bufs=3) as bpool,
        tc.tile_pool(name="psum", bufs=7, space="PSUM") as pspool,
        tc.tile_pool(name="outs", bufs=4) as opool,
        tc.tile_pool(name="tmps", bufs=4) as tpool,
    ):
        # ---------------- DMA helpers (Sync ring, consumption order) -------
        xc = []          # (t0, len, tile)
        def dma_x(c):
            t0, ln = CHUNKS_LEAD[c]
            xt = cpool.tile([P, ln, MB], FP8, tag=f"x{c}")
            nc.scalar.dma_start(xt[:], xT[:, t0 : t0 + ln, :])
            xc.append((t0, ln, xt))

        def x_slice(kt):
            for t0, ln, xt in xc:
                if t0 <= kt < t0 + ln:
                    o = kt - t0
                    return xt[:, o : o + 2]
            raise AssertionError(kt)

        hc = []          # bf16 h chunks (epilogue)
        def dma_h(cc):
            ht = cpool.tile([P, CH, MB], BF16, tag=f"h{cc}", name=f"h{cc}")
            nc.sync.dma_start(ht[:], hT[:, cc * CH : (cc + 1) * CH, :])
            hc.append(ht)

        h8c = []         # fp8 h chunks (hq path), 16 k-tiles each
        def dma_h8(cc):
            ht = cpool.tile([P, 16, MB], FP8, tag=f"h8{cc}")
            nc.sync.dma_start(ht[:], h8[:, cc * 16 : (cc + 1) * 16, :])
            h8c.append(ht)

        bcs0 = []
        def dma_b0(c):
            t0, ln = CHUNKS_LEAD[c]
            bc = cpool.tile([P, ln, NGROUP * P], FP8, tag=f"bl{c}")
            nc.sync.dma_start(bc[:], b[:, t0 : t0 + ln, :])
            bcs0.append((t0, ln, bc))

        def dma_b_group(g):
            bcs = []
            for c, (t0, ln) in enumerate(CHUNKS_MAIN):
                bc = bpool.tile(
                    [P, ln, NGROUP * P], FP8, tag=f"b{c}", name=f"b{g}_{c}"
                )
                nc.sync.dma_start(bc[:], b[:, g * KT + t0 : g * KT + t0 + ln, :])
                bcs.append((t0, ln, bc))
            return bcs

        # ---- issue order: pure matmul feed first, then hq/epilogue feeds
        # woven in at the latest moment each is needed ----
        dma_b0(0); dma_x(0); dma_b0(1); dma_x(1); dma_b0(2); dma_x(2)
        bcs1 = dma_b_group(1)
        q_sb = cpool.tile([P, KT, RANK], FP8, tag="q")
        nc.scalar.dma_start(q_sb[:], q[:])
        dma_h8(0); dma_h8(1)
        dma_h(0)
        # (x/q/pT/a_r ride the Scalar ring concurrently with the b stream)
        # pT arrives host-prepared: zeros except p^T replicated at partition
        # bases 0/32/64/96 -- the rank-4 matmul then sums the 4 column-group
        # hq partials (parked at the same bases in hq_sb) as part of its own
        # contraction, and no on-chip memset is needed.
        pT_sb = cpool.tile([P, HIDDEN], BF16, tag="pT")
        nc.scalar.dma_start(pT_sb[:], pT[:, :])
        araw = cpool.tile([P, NT], F32, tag="araw")
        nc.scalar.dma_start(araw[:], a_r[:, :])
        a1 = cpool.tile([P, NT], F32, tag="a1")
        nc.vector.tensor_scalar_add(a1[:], araw[:], 1.0)
        bcs2 = dma_b_group(2)
        dma_h(1)
        bcs3 = dma_b_group(3)
        dma_h(2)
        bcs4 = dma_b_group(4)
        dma_h(3)
        pre_bcs = {1: bcs1, 2: bcs2, 3: bcs3, 4: bcs4}

        # ---------------- compute helpers ----------------------------------
        def sub_epilogue(tn, ps, on_act=False):
            # ot = (1 + a[n]) * hT  +  2^-18 * psum   (fp16 out, host upcasts)
            sc = tpool.tile([P, MB], F16, tag="sc", name=f"sc{tn}")
            if on_act:
                nc.scalar.activation(
                    sc[:], ps[:], mybir.ActivationFunctionType.Copy, scale=INV_S
                )
            else:
                nc.vector.tensor_scalar_mul(sc[:], ps[:], INV_S)
            ot = opool.tile([P, MB], F16, tag="ot", name=f"ot{tn}")
            nc.vector.scalar_tensor_tensor(
                ot[:],
                hc[tn // CH][:, tn % CH],
                a1[:, tn : tn + 1],
                sc[:],
                mybir.AluOpType.mult,
                mybir.AluOpType.add,
            )
            nc.scalar.dma_start(outT[tn * P : (tn + 1) * P, :], ot[:])

        def rank4(tn, ps):
            nc.tensor.matmul(
                ps[:],
                pT_sb[:, tn * P : (tn + 1) * P],
                hq_sb[:],
                start=False,
                stop=True,
                skip_group_check=True,
            )

        def main_eps(g, bcs, pss, subs, start_ok=True):
            # k-outer; DoubleRow consumes 2 k-subtiles per matmul.
            for c, (t0, ln, bc) in enumerate(bcs):
                for sub in subs:
                    for u in range(ln // 2):
                        kt = t0 + 2 * u
                        nc.tensor.matmul(
                            pss[sub][:],
                            bc[:, 2 * u : 2 * u + 2, sub * P : (sub + 1) * P],
                            x_slice(kt),
                            start=False,
                            stop=False,
                            perf_mode=DR,
                            skip_group_check=True,
                        )

        def drain(g, pss, subs, tail=False):
            if tail:
                # kernel tail: no mains left, so the ACT engine can scale the
                # PSUM banks without contending with PE PSUM writes, and the
                # DVE only runs the final add per tile.
                for sub in subs:
                    rank4(g * NGROUP + sub, pss[sub])
                    sub_epilogue(g * NGROUP + sub, pss[sub], on_act=True)
            else:
                for sub in subs:
                    rank4(g * NGROUP + sub, pss[sub])
                for sub in subs:
                    sub_epilogue(g * NGROUP + sub, pss[sub])

        def group_psum(g):
            tiles = []
            for i in range(NGROUP):
                t = pspool.tile([P, MB], F32, tag="ps", name=f"ps{g}_{i}")
                # pre-zero on DVE (idle capacity, overlaps PE) so every main
                # can run start=False: accumulate-onto-zeros is correct for
                # any stale has_written state, and skips the ~220ns
                # start_tensor_calc penalty on each bank's first matmul
                nc.vector.memset(t[:], 0.0)
                tiles.append(t)
            return tiles

        # ---------------- schedule -----------------------------------------
        # hqT = q^T @ hT on the fp8 h copy, column-tiled 4-wide: k-tile kt
        # lands on col group (kt % 4), partition base 32*(kt % 4).
        pss0 = group_psum(0)
        hq_ps = pspool.tile([P, MB], F32, tag="hq", bufs=1)
        # zero the whole hq bank so the unused partition rows between the
        # column-group partials read back 0.0 (enables the single-op extract)
        nc.vector.memset(hq_ps[:], 0.0)

        def hq_chunk(cc):
            for tt in range(16):
                kt = cc * 16 + tt
                j = kt % 4
                nc.tensor.matmul(
                    hq_ps[32 * j : 32 * j + RANK, :],
                    q_sb[:, kt],
                    h8c[cc][:, tt],
                    start=False,
                    stop=(kt >= KT - 4),
                    tile_position=(0, 32 * j),
                    skip_group_check=True,
                )

        # group 0 mains (lead-in chunks)
        bl = [(t0, ln, bc) for (t0, ln, bc) in bcs0]
        main_eps(0, bl, pss0, subs=range(NGROUP))

        # group 1 mains, first 3 banks only (bank budget: g0:4 + hq:1 + 3)
        pss1 = []
        for i in range(3):
            t = pspool.tile([P, MB], F32, tag="ps", name=f"ps1_{i}")
            nc.vector.memset(t[:], 0.0)
            pss1.append(t)
        main_eps(1, bcs1, pss1, subs=[0, 1, 2])

        # hq while group-1 mains stream
        hq_chunk(0)
        hq_chunk(1)

        # extract all 4 column-group partials (plus the pre-zeroed filler
        # rows) in ONE DVE op: four slice copies serialize ~1.4us each on
        # same-tile write ordering.  hq_ps carries hq*2^17 (2^12 q * 2^5 h),
        # mains carry 2^18, so scale by 2.
        hq_sb = cpool.tile([P, MB], BF16, tag="hq_sb")
        nc.vector.tensor_scalar_mul(hq_sb[:], hq_ps[:], HQ_FIX)

        # drain group 0 (frees 4 banks), finish group 1, drain it; group
        # 1's 4th bank reuses the hq PSUM bank (free right after hq_sb)
        drain(0, pss0, range(NGROUP))
        ps1_3 = pspool.tile([P, MB], F32, tag="hq", bufs=1, name="ps1_3")
        nc.vector.memset(ps1_3[:], 0.0)
        pss1.append(ps1_3)
        main_eps(1, bcs1, pss1, subs=[3])
        drain(1, pss1, range(NGROUP))

        # groups 2..7
        for g in range(2, N_GROUPS):
            bcs = pre_bcs.get(g) or dma_b_group(g)
            pss = group_psum(g)
            main_eps(g, bcs, pss, subs=range(NGROUP))
            drain(g, pss, range(NGROUP), tail=(g == N_GROUPS - 1))

    nc.compile()
    return nc


_NC_CACHE = []


def _get_nc():
    if not _NC_CACHE:
        _NC_CACHE.append(build_bass())
    return _NC_CACHE[0]


LAST_RESULTS = []  # stash of the last BassKernelResults, for test harnesses


def make_in_maps(h, x, a_diag, p_vec, q_vec, b_mat):
    """Shard + lay out the full inputs into per-core in_maps."""
    h = np.asarray(h, dtype=np.float32)
    x = np.asarray(x, dtype=np.float32)
    a_diag = np.asarray(a_diag, dtype=np.float32)
    p_vec = np.asarray(p_vec, dtype=np.float32)
    q_vec = np.asarray(q_vec, dtype=np.float32)
    b_mat = np.asarray(b_mat, dtype=np.float32)

    bf = ml_dtypes.bfloat16
    f8 = ml_dtypes.float8_e4m3
    # b[k, n] -> b_s[p, g*KT + t, n2] with k = t*128+p, n = g*512+n2
    # (partition-major tile layout: per-partition rows are contiguous)
    b_f8 = (b_mat * np.float32(SBC)).astype(f8)
    b_s = np.ascontiguousarray(
        b_f8.reshape(KT, P, N_GROUPS, NGROUP * P).transpose(1, 2, 0, 3)
        .reshape(P, N_GROUPS * KT, NGROUP * P)
    )
    # q[k, r] -> q_s[p, t, r], fp8 scaled
    q_s = np.ascontiguousarray(
        (q_vec * np.float32(SQ)).astype(f8).reshape(KT, P, RANK).transpose(1, 0, 2)
    )
    # zeros except p^T replicated at partition bases 0/32/64/96
    pT_plane = np.zeros((P, HIDDEN), dtype=bf)
    for j in range(4):
        pT_plane[32 * j : 32 * j + RANK, :] = p_vec.T.astype(bf)
    pT_bf = np.ascontiguousarray(pT_plane)
    # a_r[p, t] = a_diag[t*128 + p]
    a_r = np.ascontiguousarray(a_diag.reshape(NT, P).T)

    in_maps = []
    for c in range(N_CORES):
        sl = slice(c * MB, (c + 1) * MB)
        hTc = h[sl].T
        x_s = np.ascontiguousarray(
            (x[sl].T * np.float32(SX)).astype(f8).reshape(KT, P, MB)
            .transpose(1, 0, 2)
        )
        h_s = np.ascontiguousarray(
            hTc.astype(bf).reshape(KT, P, MB).transpose(1, 0, 2)
        )
        h8_s = np.ascontiguousarray(
            (hTc * np.float32(SH)).astype(f8).reshape(KT, P, MB).transpose(1, 0, 2)
        )
        in_maps.append(
            {
                "b": b_s,
                "xT": x_s,
                "hT": h_s,
                "h8": h8_s,
                "q": q_s,
                "pT": pT_bf,
                "a_r": a_r,
            }
        )
    return in_maps


def _axon_device_reset():
    """Best-effort heal of a wedged axon-tunneled device (NRT_EXEC_UNIT_
    UNRECOVERABLE). No-op when the axon .so isn't present."""
    try:
        import ctypes

        lib = ctypes.CDLL("/opt/axon/libaxon_pjrt.so")
        lib.axon_reset.restype = ctypes.c_int64
        lib.axon_reset()
    except Exception:
        pass


def kernel(h, x, a_diag, p_vec, q_vec, b_mat, trace=False):
    nc = _get_nc()
    in_maps = make_in_maps(h, x, a_diag, p_vec, q_vec, b_mat)
    try:
        res = run_bass_kernel_spmd(
            nc, in_maps, core_ids=list(range(N_CORES)), trace=trace
        )
    except Exception as e:
        if "UNRECOVERABLE" not in str(e) and "UNAVAILABLE" not in str(e):
            raise
        _axon_device_reset()
        res = run_bass_kernel_spmd(
            nc, in_maps, core_ids=list(range(N_CORES)), trace=trace
        )
    LAST_RESULTS.clear()
    LAST_RESULTS.append(res)

    out = np.empty((BATCH, HIDDEN), dtype=np.float32)
    for c in range(N_CORES):
        out[c * MB : (c + 1) * MB, :] = res.results[c]["outT"].T.astype(np.float32)
    return out


# revision 24
# speedup vs baseline: 1.0932x; 1.0932x over previous
"""Trainium2 Bass kernel for the DiscretizedDPLRSSMBlock problem.

Computes, for h, x of shape [4096, 4096] (batch, hidden):

    out = h + (h * a_diag + (h @ q_vec) @ p_vec.T) + x @ b_mat        (DELTA = 1.0)
        = h * (1 + a_diag) + (h @ q_vec) @ p_vec.T + x @ b_mat

Sharding: data-parallel over the batch axis across 8 NeuronCores (512 rows
per core); a_diag/p_vec/q_vec/b_mat replicated.

Per-core kernel works in a transposed layout (hidden on partitions):
    outT[n, m] = sum_k B[k, n] * xT[k, m]        (x @ B; fp8e4 DoubleRow
                                                  matmuls, 2 k-subtiles per
                                                  instruction, 2x throughput)
               + sum_r p[n, r] * hqT[r, m]       (rank-4 term)
               + (1 + a[n]) * hT[n, m]           (per-partition scalar on DVE)

Precision plan (rel-err gate is 2e-2; measured ~1.78e-2):
  - x*2^5 and B*2^13 are cast to fp8e4 on the host (all values stay inside
    TRN e4m3's +-240 normal range); the epilogue multiplies the PSUM
    accumulator by 2^-18 (exact power of two) before adding h*(1+a).
  - hqT = q^T @ hT runs on a second fp8 copy of h (h*2^5) against q*2^12,
    column-tiled 4-wide (tile_position col groups; partials land on
    partition bases 0/32/64/96 of one PSUM bank).  The replicated-pT rows
    of the rank-4 stationary operand sum the partials inside the rank-4
    matmul itself.  The rank-4 term is ~2% of the output, so fp8 h/q
    quantization there is negligible.
  - h also arrives as bf16 -- streamed lazily, one chunk per two groups --
    for the h*(1+a) epilogue term, which needs full precision.
  - Output is fp16 (upcast on host).

DMA: one hand-ordered stream on the Sync HWDGE ring feeds the PE exactly
in consumption order (b group chunks interleaved with the x lead-in, the
fp8 h, and the lazy bf16 h); output tiles ride the Scalar ring.  All
operand tensors are host-pre-shuffled into partition-major tile layouts so
every chunk DMA reads contiguous 8-16KB runs per partition row.
"""

import numpy as np
import ml_dtypes

import concourse.mybir as mybir
import concourse.tile as tile
from concourse import bacc
from concourse.bass_utils import run_bass_kernel_spmd

HIDDEN = 4096
BATCH = 4096
RANK = 4
N_CORES = 8
MB = BATCH // N_CORES  # 512 batch rows per core
P = 128
KT = HIDDEN // P       # 32 contraction tiles
NT = HIDDEN // P       # 32 output row tiles (hidden)
NGROUP = NT // 8       # 4 n-tiles per b-column streaming group (512 cols)
CH = KT // 4           # 8 k-tiles per bf16 hT chunk (1MB)
N_GROUPS = NT // NGROUP

SX = 2.0 ** 5          # host fp8 scale on x   (|x*32|   <= ~174 < 240)
SBC = 2.0 ** 13        # host fp8 scale on b   (|b*8192| <= 128  < 240)
SH = 2.0 ** 5          # host fp8 scale on h (hq path)
SQ = 2.0 ** 12         # host fp8 scale on q   (|q*4096| <= ~91  < 240)
INV_S = float(2.0 ** -18)   # exact de-scale applied to PSUM
HQ_FIX = float(2.0 ** 1)    # hq_ps carries 2^17; mains carry 2^18

BF16 = mybir.dt.bfloat16
FP8 = mybir.dt.float8e4
F16 = mybir.dt.float16
F32 = mybir.dt.float32
DR = mybir.MatmulPerfMode.DoubleRow

# k-tile chunking.  Group 0 (and x) use small lead-in chunks so the first
# matmuls start early; steady-state b chunks are ~1MB fp8.
CHUNKS_LEAD = [(0, 4), (4, 12), (16, 16)]
CHUNKS_MAIN = [(0, 16), (16, 16)]


def build_bass():
    """Build the single-core Tile program (same program runs SPMD on all 8)."""
    nc = bacc.Bacc("TRN2", target_bir_lowering=False, debug=False)

    b = nc.dram_tensor("b", [P, N_GROUPS * KT, NGROUP * P], FP8,
                       kind="ExternalInput")
    xT = nc.dram_tensor("xT", [P, KT, MB], FP8, kind="ExternalInput")
    hT = nc.dram_tensor("hT", [P, KT, MB], BF16, kind="ExternalInput")
    h8 = nc.dram_tensor("h8", [P, KT, MB], FP8, kind="ExternalInput")
    q = nc.dram_tensor("q", [P, KT, RANK], FP8, kind="ExternalInput")
    pT = nc.dram_tensor("pT", [P, HIDDEN], BF16, kind="ExternalInput")
    a_r = nc.dram_tensor("a_r", [P, NT], F32, kind="ExternalInput")
    outT = nc.dram_tensor("outT", [HIDDEN, MB], F16, kind="ExternalOutput")

    with (
        tile.TileContext(nc) as tc,
        tc.tile_pool(name="const", bufs=1) as cpool,
        tc.tile_pool(name="bcols", bufs=3) as bpool,
        tc.tile_pool(name="psum", bufs=7, space="PSUM") as pspool,
        tc.tile_pool(name="outs", bufs=4) as opool,
        tc.tile_pool(name="tmps", bufs=4) as tpool,
    ):
        # ---------------- DMA helpers (Sync ring, consumption order) -------
        xc = []          # (t0, len, tile)
        def dma_x(c):
            t0, ln = CHUNKS_LEAD[c]
            xt = cpool.tile([P, ln, MB], FP8, tag=f"x{c}")
            nc.scalar.dma_start(xt[:], xT[:, t0 : t0 + ln, :])
            xc.append((t0, ln, xt))

        def x_slice(kt):
            for t0, ln, xt in xc:
                if t0 <= kt < t0 + ln:
                    o = kt - t0
                    return xt[:, o : o + 2]
            raise AssertionError(kt)

        hc = []          # bf16 h chunks (epilogue)
        def dma_h(cc):
            ht = cpool.tile([P, CH, MB], BF16, tag=f"h{cc}", name=f"h{cc}")
            nc.sync.dma_start(ht[:], hT[:, cc * CH : (cc + 1) * CH, :])
            hc.append(ht)

        h8c = []         # fp8 h chunks (hq path), 16 k-tiles each
        def dma_h8(cc):
            ht = cpool.tile([P, 16, MB], FP8, tag=f"h8{cc}")
            nc.sync.dma_start(ht[:], h8[:, cc * 16 : (cc + 1) * 16, :])
            h8c.append(ht)

        bcs0 = []
        def dma_b0(c):
            t0, ln = CHUNKS_LEAD[c]
            bc = cpool.tile([P, ln, NGROUP * P], FP8, tag=f"bl{c}")
            nc.sync.dma_start(bc[:], b[:, t0 : t0 + ln, :])
            bcs0.append((t0, ln, bc))

        def dma_b_group(g):
            bcs = []
            for c, (t0, ln) in enumerate(CHUNKS_MAIN):
                bc = bpool.tile(
                    [P, ln, NGROUP * P], FP8, tag=f"b{c}", name=f"b{g}_{c}"
                )
                nc.sync.dma_start(bc[:], b[:, g * KT + t0 : g * KT + t0 + ln, :])
                bcs.append((t0, ln, bc))
            return bcs

        # ---- issue order: pure matmul feed first, then hq/epilogue feeds
        # woven in at the latest moment each is needed ----
        dma_b0(0); dma_x(0); dma_b0(1); dma_x(1); dma_b0(2); dma_x(2)
        bcs1 = dma_b_group(1)
        q_sb = cpool.tile([P, KT, RANK], FP8, tag="q")
        nc.scalar.dma_start(q_sb[:], q[:])
        dma_h8(0); dma_h8(1)
        dma_h(0)
        # (x/q/pT/a_r ride the Scalar ring concurrently with the b stream)
        # pT arrives host-prepared: zeros except p^T replicated at partition
        # bases 0/32/64/96 -- the rank-4 matmul then sums the 4 column-group
        # hq partials (parked at the same bases in hq_sb) as part of its own
        # contraction, and no on-chip memset is needed.
        pT_sb = cpool.tile([P, HIDDEN], BF16, tag="pT")
        nc.scalar.dma_start(pT_sb[:], pT[:, :])
        araw = cpool.tile([P, NT], F32, tag="araw")
        nc.scalar.dma_start(araw[:], a_r[:, :])
        a1 = cpool.tile([P, NT], F32, tag="a1")
        nc.vector.tensor_scalar_add(a1[:], araw[:], 1.0)
        bcs2 = dma_b_group(2)
        dma_h(1)
        bcs3 = dma_b_group(3)
        dma_h(2)
        bcs4 = dma_b_group(4)
        dma_h(3)
        pre_bcs = {1: bcs1, 2: bcs2, 3: bcs3, 4: bcs4}

        # ---------------- compute helpers ----------------------------------
        def sub_epilogue(tn, ps, on_act=False):
            # ot = (1 + a[n]) * hT  +  2^-18 * psum   (fp16 out, host upcasts)
            sc = tpool.tile([P, MB], F16, tag="sc", name=f"sc{tn}")
            if on_act:
                nc.scalar.activation(
                    sc[:], ps[:], mybir.ActivationFunctionType.Copy, scale=INV_S
                )
            else:
                nc.vector.tensor_scalar_mul(sc[:], ps[:], INV_S)
            ot = opool.tile([P, MB], F16, tag="ot", name=f"ot{tn}")
            nc.vector.scalar_tensor_tensor(
                ot[:],
                hc[tn // CH][:, tn % CH],
                a1[:, tn : tn + 1],
                sc[:],
                mybir.AluOpType.mult,
                mybir.AluOpType.add,
            )
            nc.scalar.dma_start(outT[tn * P : (tn + 1) * P, :], ot[:])

        def rank4(tn, ps):
            nc.tensor.matmul(
                ps[:],
                pT_sb[:, tn * P : (tn + 1) * P],
                hq_sb[:],
                start=False,
                stop=True,
            )

        def main_eps(g, bcs, pss, subs, start_ok=True):
            # k-outer; DoubleRow consumes 2 k-subtiles per matmul.
            for c, (t0, ln, bc) in enumerate(bcs):
                for sub in subs:
                    for u in range(ln // 2):
                        kt = t0 + 2 * u
                        nc.tensor.matmul(
                            pss[sub][:],
                            bc[:, 2 * u : 2 * u + 2, sub * P : (sub + 1) * P],
                            x_slice(kt),
                            start=(start_ok and c == 0 and u == 0),
                            stop=False,
                            perf_mode=DR,
                        )

        def drain(g, pss, subs, tail=False):
            if tail:
                # kernel tail: no mains left, so the ACT engine can scale the
                # PSUM banks without contending with PE PSUM writes, and the
                # DVE only runs the final add per tile.
                for sub in subs:
                    rank4(g * NGROUP + sub, pss[sub])
                    sub_epilogue(g * NGROUP + sub, pss[sub], on_act=True)
            else:
                for sub in subs:
                    rank4(g * NGROUP + sub, pss[sub])
                for sub in subs:
                    sub_epilogue(g * NGROUP + sub, pss[sub])

        def group_psum(g):
            return [
                pspool.tile([P, MB], F32, tag="ps", name=f"ps{g}_{i}")
                for i in range(NGROUP)
            ]

        # ---------------- schedule -----------------------------------------
        # hqT = q^T @ hT on the fp8 h copy, column-tiled 4-wide: k-tile kt
        # lands on col group (kt % 4), partition base 32*(kt % 4).
        pss0 = group_psum(0)
        hq_ps = pspool.tile([P, MB], F32, tag="hq", bufs=1)
        # zero the whole hq bank so the unused partition rows between the
        # column-group partials read back 0.0 (enables the single-op extract)
        nc.vector.memset(hq_ps[:], 0.0)

        def hq_chunk(cc):
            for tt in range(16):
                kt = cc * 16 + tt
                j = kt % 4
                nc.tensor.matmul(
                    hq_ps[32 * j : 32 * j + RANK, :],
                    q_sb[:, kt],
                    h8c[cc][:, tt],
                    start=(kt < 4),
                    stop=(kt >= KT - 4),
                    tile_position=(0, 32 * j),
                )

        # group 0 mains (lead-in chunks)
        bl = [(t0, ln, bc) for (t0, ln, bc) in bcs0]
        main_eps(0, bl, pss0, subs=range(NGROUP))

        # group 1 mains, first 3 banks only (bank budget: g0:4 + hq:1 + 3)
        pss1 = [
            pspool.tile([P, MB], F32, tag="ps", name=f"ps1_{i}")
            for i in range(3)
        ]
        main_eps(1, bcs1, pss1, subs=[0, 1, 2])

        # hq while group-1 mains stream
        hq_chunk(0)
        hq_chunk(1)

        # extract all 4 column-group partials (plus the pre-zeroed filler
        # rows) in ONE DVE op: four slice copies serialize ~1.4us each on
        # same-tile write ordering.  hq_ps carries hq*2^17 (2^12 q * 2^5 h),
        # mains carry 2^18, so scale by 2.
        hq_sb = cpool.tile([P, MB], BF16, tag="hq_sb")
        nc.vector.tensor_scalar_mul(hq_sb[:], hq_ps[:], HQ_FIX)

        # drain group 0 (frees 4 banks), finish group 1, drain it; group
        # 1's 4th bank reuses the hq PSUM bank (free right after hq_sb)
        drain(0, pss0, range(NGROUP))
        pss1.append(pspool.tile([P, MB], F32, tag="hq", bufs=1, name="ps1_3"))
        main_eps(1, bcs1, pss1, subs=[3])
        drain(1, pss1, range(NGROUP))

        # groups 2..7
        for g in range(2, N_GROUPS):
            bcs = pre_bcs.get(g) or dma_b_group(g)
            pss = group_psum(g)
            main_eps(g, bcs, pss, subs=range(NGROUP))
            drain(g, pss, range(NGROUP), tail=(g == N_GROUPS - 1))

    nc.compile()
    return nc


_NC_CACHE = []


def _get_nc():
    if not _NC_CACHE:
        _NC_CACHE.append(build_bass())
    return _NC_CACHE[0]


LAST_RESULTS = []  # stash of the last BassKernelResults, for test harnesses


def make_in_maps(h, x, a_diag, p_vec, q_vec, b_mat):
    """Shard + lay out the full inputs into per-core in_maps."""
    h = np.asarray(h, dtype=np.float32)
    x = np.asarray(x, dtype=np.float32)
    a_diag = np.asarray(a_diag, dtype=np.float32)
    p_vec = np.asarray(p_vec, dtype=np.float32)
    q_vec = np.asarray(q_vec, dtype=np.float32)
    b_mat = np.asarray(b_mat, dtype=np.float32)

    bf = ml_dtypes.bfloat16
    f8 = ml_dtypes.float8_e4m3
    # b[k, n] -> b_s[p, g*KT + t, n2] with k = t*128+p, n = g*512+n2
    # (partition-major tile layout: per-partition rows are contiguous)
    b_f8 = (b_mat * np.float32(SBC)).astype(f8)
    b_s = np.ascontiguousarray(
        b_f8.reshape(KT, P, N_GROUPS, NGROUP * P).transpose(1, 2, 0, 3)
        .reshape(P, N_GROUPS * KT, NGROUP * P)
    )
    # q[k, r] -> q_s[p, t, r], fp8 scaled
    q_s = np.ascontiguousarray(
        (q_vec * np.float32(SQ)).astype(f8).reshape(KT, P, RANK).transpose(1, 0, 2)
    )
    # zeros except p^T replicated at partition bases 0/32/64/96
    pT_plane = np.zeros((P, HIDDEN), dtype=bf)
    for j in range(4):
        pT_plane[32 * j : 32 * j + RANK, :] = p_vec.T.astype(bf)
    pT_bf = np.ascontiguousarray(pT_plane)
    # a_r[p, t] = a_diag[t*128 + p]
    a_r = np.ascontiguousarray(a_diag.reshape(NT, P).T)

    in_maps = []
    for c in range(N_CORES):
        sl = slice(c * MB, (c + 1) * MB)
        hTc = h[sl].T
        x_s = np.ascontiguousarray(
            (x[sl].T * np.float32(SX)).astype(f8).reshape(KT, P, MB)
            .transpose(1, 0, 2)
        )
        h_s = np.ascontiguousarray(
            hTc.astype(bf).reshape(KT, P, MB).transpose(1, 0, 2)
        )
        h8_s = np.ascontiguousarray(
            (hTc * np.float32(SH)).astype(f8).reshape(KT, P, MB).transpose(1, 0, 2)
        )
        in_maps.append(
            {
                "b": b_s,
                "xT": x_s,
                "hT": h_s,
                "h8": h8_s,
                "q": q_s,
                "pT": pT_bf,
                "a_r": a_r,
            }
        )
    return in_maps


def _axon_device_reset():
    """Best-effort heal of a wedged axon-tunneled device (NRT_EXEC_UNIT_
    UNRECOVERABLE). No-op when the axon .so isn't present."""
    try:
        import ctypes

        lib = ctypes.CDLL("/opt/axon/libaxon_pjrt.so")
        lib.axon_reset.restype = ctypes.c_int64
        lib.axon_reset()
    except Exception:
        pass


def kernel(h, x, a_diag, p_vec, q_vec, b_mat, trace=False):
    nc = _get_nc()
    in_maps = make_in_maps(h, x, a_diag, p_vec, q_vec, b_mat)
    try:
        res = run_bass_kernel_spmd(
            nc, in_maps, core_ids=list(range(N_CORES)), trace=trace
        )
    except Exception as e:
        if "UNRECOVERABLE" not in str(e) and "UNAVAILABLE" not in str(e):
            raise
        _axon_device_reset()
        res = run_bass_kernel_spmd(
            nc, in_maps, core_ids=list(range(N_CORES)), trace=trace
        )
    LAST_RESULTS.clear()
    LAST_RESULTS.append(res)

    out = np.empty((BATCH, HIDDEN), dtype=np.float32)
    for c in range(N_CORES):
        out[c * MB : (c + 1) * MB, :] = res.results[c]["outT"].T.astype(np.float32)
    return out
